# revision 12
# baseline (speedup 1.0000x reference)
"""Multi-head attention (B=16, N=512, H=8, D=128) on 8 trn2 NeuronCores.

Data-parallel over batch: each core handles 2 batches. Per core, per batch:
  qT/kT projections in [d, token] layout; scores computed transposed
  sT[m, n] so attention*V needs no transposes.
  dist is pre-transposed on the host with the column (key) mask folded in
  (-1e9 rows) and shipped bf16:  distc[m, n] = dist[n, m] + cm[m].
  E = exp(distc) is computed once per batch (one big ACT op) and shared by
  all 8 heads: p = exp(s) * E.
  k-bias is dropped entirely (softmax over keys is invariant to per-query
  shifts); q-bias rides the qT PSUM->SBUF copy; v-bias folds into the
  output bias on the host (softmax rows sum to 1): bo' = bo + Wo^T bv.
  Softmax denominators come from a PE ones-matmul with a [128,128] ones
  stationary, which directly yields the rowsum broadcast across all 128
  partitions; yT = y_raw / rowsum is a single DVE divide (no reciprocal,
  no DRAM roundtrip).  The final row mask is applied as a per-partition
  scale on the output-transpose copies.
"""

import sys

sys.path.insert(0, "/opt/trn_rl_repo")

import numpy as np
from contextlib import ExitStack

import ml_dtypes
import concourse.bass as bass
import concourse.bacc as bacc
import concourse.tile as tile
from concourse import mybir
from concourse.masks import make_identity

B, N, H, D = 16, 512, 8, 128
NCORES = 8
BPC = B // NCORES  # batches per core
NT = N // 128  # 128-token tiles per batch
F32 = mybir.dt.float32
F32R = mybir.dt.float32r
BF16 = mybir.dt.bfloat16

N_WARMUP_MM = 10  # dummy matmuls to lift the HAM clock gate early


def build_kernel():
    nc = bacc.Bacc("TRN2", target_bir_lowering=False, debug=False)

    #   xm_in  [BPC, 128, 516] f32: cols 0-511 x as [p, nt, d]; 512-515 maskT
    #   dc_in  [BPC, 128, NT*N] bf16: distT + colmask as [m_in_tile, mt, n]
    #   wb_in  [128, 4224] bf16: wq' | wk | wv | wo(k-major, head, d_out) | row0: bo_eff
    #   wf_in  [128, 8] f32: bq'
    xm_d = nc.declare_dram_parameter("xm_in", [BPC, 128, 516], F32, isOutput=False).ap()
    dc_d = nc.declare_dram_parameter("dc_in", [BPC, 128, NT * N], BF16, isOutput=False).ap()
    wb_d = nc.declare_dram_parameter("wb_in", [D, 4 * H * D + 128], BF16, isOutput=False).ap()
    wf_d = nc.declare_dram_parameter("wf_in", [D, 8], F32R, isOutput=False).ap()
    y_d = nc.declare_dram_parameter("y_out", [BPC, N, D], F32, isOutput=True).ap()

    with tile.TileContext(nc) as tc, ExitStack() as ctx:
        # ---------------- pools ----------------
        consts = ctx.enter_context(tc.tile_pool(name="consts", bufs=1))
        stage = ctx.enter_context(tc.tile_pool(name="stage", bufs=2))
        dcp = ctx.enter_context(tc.tile_pool(name="dcp", bufs=2))
        epool = ctx.enter_context(tc.tile_pool(name="epool", bufs=2))
        xpool = ctx.enter_context(tc.tile_pool(name="xpool", bufs=2))
        qkp = ctx.enter_context(tc.tile_pool(name="qkp", bufs=18))
        vpool = ctx.enter_context(tc.tile_pool(name="vpool", bufs=8))
        espool = ctx.enter_context(tc.tile_pool(name="espool", bufs=3))
        ppool = ctx.enter_context(tc.tile_pool(name="ppool", bufs=3))
        ypool = ctx.enter_context(tc.tile_pool(name="ypool", bufs=10))
        rpool = ctx.enter_context(tc.tile_pool(name="rpool", bufs=2))
        opool = ctx.enter_context(tc.tile_pool(name="opool", bufs=2))

        # PSUM budget (8 banks): scores pairs 2x2 + py 1 + rB 1 + pso 1 + pt 1
        ps_s = ctx.enter_context(tc.tile_pool(name="ps_s", bufs=2, space="PSUM"))
        ps_y = ctx.enter_context(tc.tile_pool(name="ps_y", bufs=1, space="PSUM"))
        ps_r = ctx.enter_context(tc.tile_pool(name="ps_r", bufs=1, space="PSUM"))
        ps_o = ctx.enter_context(tc.tile_pool(name="ps_o", bufs=1, space="PSUM"))
        ps_t = ctx.enter_context(tc.tile_pool(name="ps_t", bufs=1, space="PSUM"))

        # ---------------- prefetch DMAs (priority order) ----------------
        xms = []
        xm0 = stage.tile([128, 516], F32, tag="xm", name="xm0")
        nc.sync.dma_start(out=xm0, in_=xm_d[0])
        xms.append(xm0)
        wb = consts.tile([128, 4 * H * D + 128], BF16, tag="wb")
        nc.sync.dma_start(out=wb, in_=wb_d)
        dcs = []
        dc0 = dcp.tile([128, NT * N], BF16, tag="dc", name="dc0")
        nc.sync.dma_start(out=dc0, in_=dc_d[0])
        dcs.append(dc0)
        xm1 = stage.tile([128, 516], F32, tag="xm", name="xm1")
        nc.sync.dma_start(out=xm1, in_=xm_d[1])
        xms.append(xm1)
        wf = consts.tile([128, 8], F32R, tag="wf")
        nc.sync.dma_start(out=wf, in_=wf_d)
        dc1 = dcp.tile([128, NT * N], BF16, tag="dc", name="dc1")
        nc.sync.dma_start(out=dc1, in_=dc_d[1])
        dcs.append(dc1)

        wq_sb = wb[:, 0:1024]
        wk_sb = wb[:, 1024:2048]
        wv_sb = wb[:, 2048:3072]
        wo_sb = wb[:, 3072:4096].rearrange("k (h d) -> k h d", h=H)
        bo_row = wb[0:1, 4096:4224]
        bq_sb = wf.bitcast(F32)

        ident = consts.tile([128, 128], F32, tag="ident")
        make_identity(nc, ident)
        ones_bf = consts.tile([128, 128], BF16, tag="ones")
        nc.vector.memset(ones_bf, 1.0)
        ones_row = consts.tile([1, N], BF16, tag="onesrow")
        nc.vector.memset(ones_row, 1.0)
        warm = consts.tile([128, 512], BF16, tag="warm")
        nc.vector.memset(warm, 0.0)

        # ---------------- HAM warmup: dummy matmuls to lift the clock gate ----------------
        for w in range(N_WARMUP_MM):
            psw = ps_s.tile([128, 1024], F32, tag="ps_s", name=f"warm{w}")
            nc.tensor.matmul(psw[:, 0:512], ones_bf, warm)

        def prologue(b):
            """xT transpose, E, v/q/k projections for batch b."""
            xm = xms[b]
            x_nat = xm[:, 0:512].rearrange("p (t d) -> p t d", t=NT)

            # E = exp(distc) for all 4 m-tiles in one ACT op
            E = epool.tile([128, NT * N], BF16, tag="E", name=f"E{b}")
            nc.scalar.activation(
                out=E, in_=dcs[b], func=mybir.ActivationFunctionType.Exp
            )

            # xT [d, n] bf16: 4 transposes into one PSUM bank, one copy out
            xT = xpool.tile([128, N], BF16, tag="xT", name=f"xT{b}")
            pst = ps_t.tile([128, 512], F32, tag="pst", name=f"pxt{b}")
            for nt in range(NT):
                nc.tensor.transpose(pst[:, nt * 128:(nt + 1) * 128], x_nat[:, nt, :], ident)
            nc.vector.tensor_copy(out=xT, in_=pst)

            # v projection -> vv[mt] [m, h*d] bf16 (bias folded out on host)
            vv = []
            for mt in range(NT):
                vmt = vpool.tile([128, H * D], BF16, tag="vv", name=f"v{b}_{mt}")
                for dh in range(2):
                    psv = ps_s.tile([128, 1024], F32, tag="ps_s", name=f"psv{b}_{mt}_{dh}")
                    nc.tensor.matmul(
                        psv[:, 0:512],
                        xT[:, mt * 128:(mt + 1) * 128],
                        wv_sb[:, dh * 512:(dh + 1) * 512],
                    )
                    nc.vector.tensor_copy(
                        out=vmt[:, dh * 512:(dh + 1) * 512], in_=psv[:, 0:512]
                    )
                vv.append(vmt)

            # q/k projections -> qT/kT [d, n] bf16 per head
            qT, kT = [], []
            for h in range(H):
                psq = ps_s.tile([128, 1024], F32, tag="ps_s", name=f"psq{b}_{h}")
                nc.tensor.matmul(psq[:, 0:512], wq_sb[:, h * D:(h + 1) * D], xT)
                qTh = qkp.tile([128, N], BF16, tag="qT", name=f"qT{b}_{h}")
                if h % 2 == 0:
                    nc.scalar.activation(
                        out=qTh, in_=psq[:, 0:512],
                        func=mybir.ActivationFunctionType.Identity,
                        bias=bq_sb[:, h:h + 1],
                    )
                else:
                    nc.vector.tensor_scalar_add(
                        out=qTh, in0=psq[:, 0:512], scalar1=bq_sb[:, h:h + 1]
                    )
                qT.append(qTh)
                psk = ps_s.tile([128, 1024], F32, tag="ps_s", name=f"psk{b}_{h}")
                nc.tensor.matmul(psk[:, 0:512], wk_sb[:, h * D:(h + 1) * D], xT)
                kTh = qkp.tile([128, N], BF16, tag="kT", name=f"kT{b}_{h}")
                nc.vector.tensor_copy(out=kTh, in_=psk[:, 0:512])
                kT.append(kTh)
            return E, vv, qT, kT

        def head_block(b, h, E, vv, qT, kT):
            """scores + exp + *E + rowsum-broadcast + attnV + divide for one head."""
            p = ppool.tile([128, NT * N], BF16, tag="p", name=f"p{b}_{h}")
            for mtp in range(2):
                pss = ps_s.tile([128, 1024], F32, tag="ps_s", name=f"pss{b}_{h}_{mtp}")
                for j in range(2):
                    mt = 2 * mtp + j
                    nc.tensor.matmul(
                        pss[:, j * 512:(j + 1) * 512],
                        kT[h][:, mt * 128:(mt + 1) * 128],
                        qT[h],
                    )
                es = espool.tile([128, 1024], BF16, tag="es", name=f"es{b}_{h}_{mtp}")
                nc.scalar.activation(
                    out=es, in_=pss, func=mybir.ActivationFunctionType.Exp
                )
                # p = es * E  (GpSimd: the only SBUF-only elementwise work)
                nc.gpsimd.tensor_tensor(
                    out=p[:, mtp * 1024:(mtp + 1) * 1024],
                    in0=es,
                    in1=E[:, mtp * 1024:(mtp + 1) * 1024],
                    op=mybir.AluOpType.mult,
                )

            # rowsum broadcast across partitions: rB[j, n] = sum_m p[m, n]
            rB = ps_r.tile([128, N], F32, tag="rB", name=f"rB{b}_{h}")
            for mt in range(NT):
                nc.tensor.matmul(
                    rB, ones_bf, p[:, mt * 512:(mt + 1) * 512],
                    start=(mt == 0), stop=(mt == NT - 1),
                )
            rb_sb = rpool.tile([128, N], F32, tag="rb", name=f"rb{b}_{h}")
            nc.vector.reciprocal_approx_fast(out=rb_sb, in_=rB)
            # attnV: yT_raw[d, n]
            py = ps_y.tile([128, N], F32, tag="py", name=f"py{b}_{h}")
            for mt in range(NT):
                nc.tensor.matmul(
                    py,
                    vv[mt][:, h * D:(h + 1) * D],
                    p[:, mt * 512:(mt + 1) * 512],
                    start=(mt == 0), stop=(mt == NT - 1),
                )
            yTn = ypool.tile([128, N], BF16, tag="yTn", name=f"yTn{b}_{h}")
            nc.vector.tensor_tensor(
                out=yTn, in0=py, in1=rb_sb, op=mybir.AluOpType.mult
            )
            return yTn

        def tail(b, yTns, maskT):
            """output projection, bias, transpose back with row-mask, store."""
            pso = ps_o.tile([128, N], F32, tag="pso", name=f"pso{b}")
            nc.tensor.matmul(pso, bo_row, ones_row, start=True, stop=False)
            for h in range(H):
                nc.tensor.matmul(
                    pso, wo_sb[:, h, :], yTns[h],
                    start=False, stop=(h == H - 1),
                )
            oT = stage.tile([128, N], F32, tag="oT", name=f"oT{b}")
            nc.scalar.copy(out=oT, in_=pso)

            o_nat = opool.tile([128, NT, D], F32, tag="o_nat", name=f"on{b}")
            for nt in range(NT):
                pst = ps_t.tile([128, 512], F32, tag="pst", name=f"pot{b}_{nt}")
                nc.tensor.transpose(pst[:, 0:128], oT[:, nt * 128:(nt + 1) * 128], ident)
                # row mask as per-partition scale during the copy
                nc.scalar.mul(out=o_nat[:, nt, :], in_=pst[:, 0:128], mul=maskT[:, nt:nt + 1])
            nc.sync.dma_start(
                out=y_d[b].rearrange("(t p) d -> p t d", p=128), in_=o_nat
            )

        # ---------------- schedule ----------------
        E0, vv0, qT0, kT0 = prologue(0)
        maskT0 = xms[0][:, 512:516]
        yTns0 = []
        for h in range(H):
            yTns0.append(head_block(0, h, E0, vv0, qT0, kT0))
        E1, vv1, qT1, kT1 = prologue(1)
        maskT1 = xms[1][:, 512:516]
        tail(0, yTns0, maskT0)
        yTns1 = []
        for h in range(H):
            yTns1.append(head_block(1, h, E1, vv1, qT1, kT1))
        tail(1, yTns1, maskT1)

    nc.compile()
    return nc


_NC_CACHE = None


def _get_nc():
    global _NC_CACHE
    if _NC_CACHE is None:
        _NC_CACHE = build_kernel()
    return _NC_CACHE


def kernel(x, dist, mask, Wq, bq, Wk, bk, Wv, bv, Wo, bo, **kw):
    from concourse.bass_utils import run_bass_kernel_spmd

    x = np.ascontiguousarray(np.asarray(x, dtype=np.float32))
    dist = np.ascontiguousarray(np.asarray(dist, dtype=np.float32))
    mask = np.ascontiguousarray(np.asarray(mask, dtype=np.float32))
    Wq = np.asarray(Wq, np.float32)
    Wk = np.asarray(Wk, np.float32)
    Wv = np.asarray(Wv, np.float32)
    Wo = np.asarray(Wo, np.float32)
    bq = np.asarray(bq, np.float32)
    bv = np.asarray(bv, np.float32)
    bo = np.asarray(bo, np.float32)

    scale = np.float32(D) ** np.float32(-0.5)
    # wb blob [128, 4224] bf16: wq' | wk | wv | wo (as [k, h, d_out]) | row0: bo_eff
    wo_r = Wo.reshape(H, D, D).transpose(1, 0, 2).reshape(D, H * D)
    wb = np.zeros((D, 4 * H * D + 128), np.float32)
    wb[:, 0:4096] = np.concatenate([Wq * scale, Wk, Wv, wo_r], axis=1)
    wb[0, 4096:4224] = bo + bv @ Wo
    wb = wb.astype(ml_dtypes.bfloat16)
    # wf blob [128, 8] f32: bq'
    wf = (bq * scale).reshape(H, D).T.astype(np.float32)
    wf = np.ascontiguousarray(wf)
    # xm [B, 128, 516] f32: x as [p, nt*d] | maskT
    xm = np.zeros((B, 128, 516), np.float32)
    xm[:, :, 0:512] = x.reshape(B, NT, 128, D).transpose(0, 2, 1, 3).reshape(B, 128, 512)
    xm[:, :, 512:516] = mask.reshape(B, NT, 128).transpose(0, 2, 1)
    # dc [B, 128, NT*N] bf16: dist transposed + column(key) mask, [m_in_tile, mt, n]
    cm = (mask - 1.0) * np.float32(1e9)  # [B, N] over keys m
    distT = dist.transpose(0, 2, 1) + cm[:, :, None]  # [B, m, n]
    dc = np.ascontiguousarray(
        distT.reshape(B, NT, 128, N).transpose(0, 2, 1, 3).reshape(B, 128, NT * N)
    ).astype(ml_dtypes.bfloat16)

    nc = _get_nc()
    in_maps = []
    for c in range(NCORES):
        sl = slice(c * BPC, (c + 1) * BPC)
        in_maps.append(
            {
                "xm_in": np.ascontiguousarray(xm[sl]),
                "dc_in": np.ascontiguousarray(dc[sl]),
                "wb_in": wb,
                "wf_in": wf,
            }
        )
    res = run_bass_kernel_spmd(nc, in_maps, core_ids=list(range(NCORES)), **kw)
    global LAST_RESULT
    LAST_RESULT = res
    out = np.concatenate([res.results[c]["y_out"] for c in range(NCORES)], axis=0)
    return out


LAST_RESULT = None


if __name__ == "__main__":
    nc = build_kernel()
    print("kernel built ok")


# revision 15
# speedup vs baseline: 1.4086x; 1.4086x over previous
"""Multi-head attention (B=16, N=512, H=8, D=128) on 8 trn2 NeuronCores.

Data-parallel over batch: each core handles 2 batches. Per core, per batch:
  qT/kT projections in [d, token] layout; scores computed transposed
  sT[m, n] so attention*V needs no transposes.
  dist is pre-transposed on the host with the column (key) mask folded in
  (-1e9 rows) and shipped bf16:  distc[m, n] = dist[n, m] + cm[m].
  E = exp(distc) is computed once per batch (one big ACT op) and shared by
  all 8 heads: p = exp(s) * E.
  k-bias is dropped entirely (softmax over keys is invariant to per-query
  shifts); q-bias rides the qT PSUM->SBUF copy; v-bias folds into the
  output bias on the host (softmax rows sum to 1): bo' = bo + Wo^T bv.
  Softmax denominators come from a PE ones-matmul with a [128,128] ones
  stationary, which directly yields the rowsum broadcast across all 128
  partitions; yT = y_raw / rowsum is a single DVE divide (no reciprocal,
  no DRAM roundtrip).  The final row mask is applied as a per-partition
  scale on the output-transpose copies.
"""

import sys

sys.path.insert(0, "/opt/trn_rl_repo")

import numpy as np
from contextlib import ExitStack

import ml_dtypes
import concourse.bass as bass
import concourse.bacc as bacc
import concourse.tile as tile
from concourse import mybir
from concourse.masks import make_identity

B, N, H, D = 16, 512, 8, 128
NCORES = 8
BPC = B // NCORES  # batches per core
NT = N // 128  # 128-token tiles per batch
F32 = mybir.dt.float32
F32R = mybir.dt.float32r
BF16 = mybir.dt.bfloat16

N_WARMUP_MM = 10  # dummy matmuls to lift the HAM clock gate early


def build_kernel():
    nc = bacc.Bacc("TRN2", target_bir_lowering=False, debug=False)

    #   xm_in  [BPC, 128, 516] f32: cols 0-511 x as [p, nt, d]; 512-515 maskT
    #   dc_in  [BPC, 128, NT*N] bf16: distT + colmask as [m_in_tile, mt, n]
    #   wb_in  [128, 4224] bf16: wq' | wk | wv | wo(k-major, head, d_out) | row0: bo_eff
    #   wf_in  [128, 8] f32: bq'
    xm_d = nc.declare_dram_parameter("xm_in", [BPC, 128, 516], F32, isOutput=False).ap()
    dc_d = nc.declare_dram_parameter("dc_in", [BPC, 128, NT * N], BF16, isOutput=False).ap()
    wb_d = nc.declare_dram_parameter("wb_in", [D, 4 * H * D + 128], BF16, isOutput=False).ap()
    wf_d = nc.declare_dram_parameter("wf_in", [D, 8], F32R, isOutput=False).ap()
    y_d = nc.declare_dram_parameter("y_out", [BPC, N, D], F32, isOutput=True).ap()

    with tile.TileContext(nc) as tc, ExitStack() as ctx:
        # ---------------- pools ----------------
        consts = ctx.enter_context(tc.tile_pool(name="consts", bufs=1))
        stage = ctx.enter_context(tc.tile_pool(name="stage", bufs=2))
        dcp = ctx.enter_context(tc.tile_pool(name="dcp", bufs=2))
        epool = ctx.enter_context(tc.tile_pool(name="epool", bufs=2))
        xpool = ctx.enter_context(tc.tile_pool(name="xpool", bufs=2))
        qkp = ctx.enter_context(tc.tile_pool(name="qkp", bufs=18))
        vpool = ctx.enter_context(tc.tile_pool(name="vpool", bufs=8))
        espool = ctx.enter_context(tc.tile_pool(name="espool", bufs=3))
        ppool = ctx.enter_context(tc.tile_pool(name="ppool", bufs=3))
        ypool = ctx.enter_context(tc.tile_pool(name="ypool", bufs=10))
        rpool = ctx.enter_context(tc.tile_pool(name="rpool", bufs=2))
        opool = ctx.enter_context(tc.tile_pool(name="opool", bufs=2))

        # PSUM budget (8 banks): scores pairs 2x2 + py 1 + rB 1 + pso 1 + pt 1
        ps_s = ctx.enter_context(tc.tile_pool(name="ps_s", bufs=2, space="PSUM"))
        ps_y = ctx.enter_context(tc.tile_pool(name="ps_y", bufs=1, space="PSUM"))
        ps_r = ctx.enter_context(tc.tile_pool(name="ps_r", bufs=1, space="PSUM"))
        ps_o = ctx.enter_context(tc.tile_pool(name="ps_o", bufs=1, space="PSUM"))
        ps_t = ctx.enter_context(tc.tile_pool(name="ps_t", bufs=1, space="PSUM"))

        # ---------------- prefetch DMAs (priority order) ----------------
        xms = []
        xm0 = stage.tile([128, 516], F32, tag="xm", name="xm0")
        nc.sync.dma_start(out=xm0, in_=xm_d[0])
        xms.append(xm0)
        wb = consts.tile([128, 4 * H * D + 128], BF16, tag="wb")
        nc.sync.dma_start(out=wb, in_=wb_d)
        dcs = []
        dc0 = dcp.tile([128, NT * N], BF16, tag="dc", name="dc0")
        nc.sync.dma_start(out=dc0, in_=dc_d[0])
        dcs.append(dc0)
        xm1 = stage.tile([128, 516], F32, tag="xm", name="xm1")
        nc.sync.dma_start(out=xm1, in_=xm_d[1])
        xms.append(xm1)
        wf = consts.tile([128, 8], F32R, tag="wf")
        nc.sync.dma_start(out=wf, in_=wf_d)
        dc1 = dcp.tile([128, NT * N], BF16, tag="dc", name="dc1")
        nc.sync.dma_start(out=dc1, in_=dc_d[1])
        dcs.append(dc1)

        wq_sb = wb[:, 0:1024]
        wk_sb = wb[:, 1024:2048]
        wv_sb = wb[:, 2048:3072]
        wo_sb = wb[:, 3072:4096].rearrange("k (h d) -> k h d", h=H)
        bo_row = wb[0:1, 4096:4224]
        bq_sb = wf.bitcast(F32)

        ident = consts.tile([128, 128], F32, tag="ident")
        make_identity(nc, ident)
        ones_bf = consts.tile([128, 128], BF16, tag="ones")
        nc.vector.memset(ones_bf, 1.0)
        ones_row = consts.tile([1, N], BF16, tag="onesrow")
        nc.vector.memset(ones_row, 1.0)
        warm = consts.tile([128, 512], BF16, tag="warm")
        nc.vector.memset(warm, 0.0)

        # ---------------- HAM warmup: dummy matmuls to lift the clock gate ----------------
        for w in range(N_WARMUP_MM):
            psw = ps_s.tile([128, 1024], F32, tag="ps_s", name=f"warm{w}")
            nc.tensor.matmul(psw[:, 0:512], ones_bf, warm)

        def prologue(b):
            """xT transpose, E, v/q/k projections for batch b."""
            xm = xms[b]
            x_nat = xm[:, 0:512].rearrange("p (t d) -> p t d", t=NT)

            # E = exp(distc) for all 4 m-tiles in one ACT op
            E = epool.tile([128, NT * N], BF16, tag="E", name=f"E{b}")
            nc.scalar.activation(
                out=E, in_=dcs[b], func=mybir.ActivationFunctionType.Exp
            )

            # xT [d, n] bf16: 4 transposes into one PSUM bank, one copy out
            xT = xpool.tile([128, N], BF16, tag="xT", name=f"xT{b}")
            pst = ps_t.tile([128, 512], F32, tag="pst", name=f"pxt{b}")
            for nt in range(NT):
                nc.tensor.transpose(pst[:, nt * 128:(nt + 1) * 128], x_nat[:, nt, :], ident)
            nc.vector.tensor_copy(out=xT, in_=pst)

            # v projection -> vv[mt] [m, h*d] bf16 (bias folded out on host)
            vv = []
            for mt in range(NT):
                vmt = vpool.tile([128, H * D], BF16, tag="vv", name=f"v{b}_{mt}")
                for dh in range(2):
                    psv = ps_s.tile([128, 1024], F32, tag="ps_s", name=f"psv{b}_{mt}_{dh}")
                    nc.tensor.matmul(
                        psv[:, 0:512],
                        xT[:, mt * 128:(mt + 1) * 128],
                        wv_sb[:, dh * 512:(dh + 1) * 512],
                    )
                    nc.vector.tensor_copy(
                        out=vmt[:, dh * 512:(dh + 1) * 512], in_=psv[:, 0:512]
                    )
                vv.append(vmt)

            # q/k projections -> qT/kT [d, n] bf16 per head
            qT, kT = [], []
            for h in range(H):
                psq = ps_s.tile([128, 1024], F32, tag="ps_s", name=f"psq{b}_{h}")
                nc.tensor.matmul(psq[:, 0:512], wq_sb[:, h * D:(h + 1) * D], xT)
                qTh = qkp.tile([128, N], BF16, tag="qT", name=f"qT{b}_{h}")
                nc.scalar.activation(
                    out=qTh, in_=psq[:, 0:512],
                    func=mybir.ActivationFunctionType.Identity,
                    bias=bq_sb[:, h:h + 1],
                )
                qT.append(qTh)
                psk = ps_s.tile([128, 1024], F32, tag="ps_s", name=f"psk{b}_{h}")
                nc.tensor.matmul(psk[:, 0:512], wk_sb[:, h * D:(h + 1) * D], xT)
                kTh = qkp.tile([128, N], BF16, tag="kT", name=f"kT{b}_{h}")
                nc.vector.tensor_copy(out=kTh, in_=psk[:, 0:512])
                kT.append(kTh)
            return E, vv, qT, kT

        def head_block(b, h, E, vv, qT, kT):
            """scores + exp + *E + rowsum-broadcast + attnV + divide for one head."""
            p = ppool.tile([128, NT * N], BF16, tag="p", name=f"p{b}_{h}")
            for mtp in range(2):
                pss = ps_s.tile([128, 1024], F32, tag="ps_s", name=f"pss{b}_{h}_{mtp}")
                for j in range(2):
                    mt = 2 * mtp + j
                    nc.tensor.matmul(
                        pss[:, j * 512:(j + 1) * 512],
                        kT[h][:, mt * 128:(mt + 1) * 128],
                        qT[h],
                    )
                es = espool.tile([128, 1024], BF16, tag="es", name=f"es{b}_{h}_{mtp}")
                nc.scalar.activation(
                    out=es, in_=pss, func=mybir.ActivationFunctionType.Exp
                )
                # p = es * E  (pair 0 on DVE, pair 1 on GpSimd: both overlap
                # within one head's PE window; GpSimd is ~2.5x slower)
                eng = nc.vector if mtp == 0 else nc.gpsimd
                eng.tensor_tensor(
                    out=p[:, mtp * 1024:(mtp + 1) * 1024],
                    in0=es,
                    in1=E[:, mtp * 1024:(mtp + 1) * 1024],
                    op=mybir.AluOpType.mult,
                )

            # rowsum broadcast across partitions: rB[j, n] = sum_m p[m, n]
            rB = ps_r.tile([128, N], F32, tag="rB", name=f"rB{b}_{h}")
            for mt in range(NT):
                nc.tensor.matmul(
                    rB, ones_bf, p[:, mt * 512:(mt + 1) * 512],
                    start=(mt == 0), stop=(mt == NT - 1),
                )
            rb_sb = rpool.tile([128, N], F32, tag="rb", name=f"rb{b}_{h}")
            nc.vector.reciprocal_approx_fast(out=rb_sb, in_=rB)
            # attnV: yT_raw[d, n]
            py = ps_y.tile([128, N], F32, tag="py", name=f"py{b}_{h}")
            for mt in range(NT):
                nc.tensor.matmul(
                    py,
                    vv[mt][:, h * D:(h + 1) * D],
                    p[:, mt * 512:(mt + 1) * 512],
                    start=(mt == 0), stop=(mt == NT - 1),
                )
            yTn = ypool.tile([128, N], BF16, tag="yTn", name=f"yTn{b}_{h}")
            nc.vector.tensor_tensor(
                out=yTn, in0=py, in1=rb_sb, op=mybir.AluOpType.mult
            )
            return yTn

        def tail(b, yTns, maskT):
            """output projection, bias, transpose back with row-mask, store."""
            pso = ps_o.tile([128, N], F32, tag="pso", name=f"pso{b}")
            nc.tensor.matmul(pso, bo_row, ones_row, start=True, stop=False)
            for h in range(H):
                nc.tensor.matmul(
                    pso, wo_sb[:, h, :], yTns[h],
                    start=False, stop=(h == H - 1),
                )
            oT = stage.tile([128, N], F32, tag="oT", name=f"oT{b}")
            nc.scalar.copy(out=oT, in_=pso)

            o_nat = opool.tile([128, NT, D], F32, tag="o_nat", name=f"on{b}")
            for nt in range(NT):
                pst = ps_t.tile([128, 512], F32, tag="pst", name=f"pot{b}_{nt}")
                nc.tensor.transpose(pst[:, 0:128], oT[:, nt * 128:(nt + 1) * 128], ident)
                # row mask as per-partition scale during the copy
                nc.vector.tensor_scalar_mul(
                    out=o_nat[:, nt, :], in0=pst[:, 0:128], scalar1=maskT[:, nt:nt + 1]
                )
            nc.sync.dma_start(
                out=y_d[b].rearrange("(t p) d -> p t d", p=128), in_=o_nat
            )

        # ---------------- schedule ----------------
        E0, vv0, qT0, kT0 = prologue(0)
        maskT0 = xms[0][:, 512:516]
        yTns0 = []
        for h in range(H):
            yTns0.append(head_block(0, h, E0, vv0, qT0, kT0))
        E1, vv1, qT1, kT1 = prologue(1)
        maskT1 = xms[1][:, 512:516]
        tail(0, yTns0, maskT0)
        yTns1 = []
        for h in range(H):
            yTns1.append(head_block(1, h, E1, vv1, qT1, kT1))
        tail(1, yTns1, maskT1)

    nc.compile()
    return nc


_NC_CACHE = None


def _get_nc():
    global _NC_CACHE
    if _NC_CACHE is None:
        _NC_CACHE = build_kernel()
    return _NC_CACHE


def kernel(x, dist, mask, Wq, bq, Wk, bk, Wv, bv, Wo, bo, **kw):
    from concourse.bass_utils import run_bass_kernel_spmd

    x = np.ascontiguousarray(np.asarray(x, dtype=np.float32))
    dist = np.ascontiguousarray(np.asarray(dist, dtype=np.float32))
    mask = np.ascontiguousarray(np.asarray(mask, dtype=np.float32))
    Wq = np.asarray(Wq, np.float32)
    Wk = np.asarray(Wk, np.float32)
    Wv = np.asarray(Wv, np.float32)
    Wo = np.asarray(Wo, np.float32)
    bq = np.asarray(bq, np.float32)
    bv = np.asarray(bv, np.float32)
    bo = np.asarray(bo, np.float32)

    scale = np.float32(D) ** np.float32(-0.5)
    # wb blob [128, 4224] bf16: wq' | wk | wv | wo (as [k, h, d_out]) | row0: bo_eff
    wo_r = Wo.reshape(H, D, D).transpose(1, 0, 2).reshape(D, H * D)
    wb = np.zeros((D, 4 * H * D + 128), np.float32)
    wb[:, 0:4096] = np.concatenate([Wq * scale, Wk, Wv, wo_r], axis=1)
    wb[0, 4096:4224] = bo + bv @ Wo
    wb = wb.astype(ml_dtypes.bfloat16)
    # wf blob [128, 8] f32: bq'
    wf = (bq * scale).reshape(H, D).T.astype(np.float32)
    wf = np.ascontiguousarray(wf)
    # xm [B, 128, 516] f32: x as [p, nt*d] | maskT
    xm = np.zeros((B, 128, 516), np.float32)
    xm[:, :, 0:512] = x.reshape(B, NT, 128, D).transpose(0, 2, 1, 3).reshape(B, 128, 512)
    xm[:, :, 512:516] = mask.reshape(B, NT, 128).transpose(0, 2, 1)
    # dc [B, 128, NT*N] bf16: dist transposed + column(key) mask, [m_in_tile, mt, n]
    cm = (mask - 1.0) * np.float32(1e9)  # [B, N] over keys m
    distT = dist.transpose(0, 2, 1) + cm[:, :, None]  # [B, m, n]
    dc = np.ascontiguousarray(
        distT.reshape(B, NT, 128, N).transpose(0, 2, 1, 3).reshape(B, 128, NT * N)
    ).astype(ml_dtypes.bfloat16)

    nc = _get_nc()
    in_maps = []
    for c in range(NCORES):
        sl = slice(c * BPC, (c + 1) * BPC)
        in_maps.append(
            {
                "xm_in": np.ascontiguousarray(xm[sl]),
                "dc_in": np.ascontiguousarray(dc[sl]),
                "wb_in": wb,
                "wf_in": wf,
            }
        )
    res = run_bass_kernel_spmd(nc, in_maps, core_ids=list(range(NCORES)), **kw)
    global LAST_RESULT
    LAST_RESULT = res
    out = np.concatenate([res.results[c]["y_out"] for c in range(NCORES)], axis=0)
    return out


LAST_RESULT = None


if __name__ == "__main__":
    nc = build_kernel()
    print("kernel built ok")
